# revision 20
# baseline (speedup 1.0000x reference)
"""Trainium2 Bass kernel for a single attention head (nn_AttentionHead).

Problem: B=16, S=2048, W=768, H=64.
  Q = input @ Wq + bq ; K = input @ Wk + bk ; V = input @ Wv + bv
  scores = Q K^T / sqrt(H), key-padding mask, softmax, out = attn @ V.

Sharding: data-parallel over batch across 8 cores (2 samples per core).

Design (per core). Two cost facts drive it: TensorE matmul time
depends only on moving columns (contraction depth is free), and ScalarE
exp costs 0.83 ns per score-matrix column. Both scale with the number of
KEY tiles, and masked keys (about half: exp == 0 exactly) contribute
nothing — so the host compacts each sample's keys to the valid subset
(padded to whole 128-key tiles; pad keys get a -100 exp bias so they
are exactly zero, making compaction bit-equivalent).

  1. Host packs X^T bf16 [B, P, NW, S] for the Q pass, the compacted
     X_kv^T bf16 [B, P, NW, SKV] for the K/V pass, stationaries
     Wq / [Wk|Wv], biases, and the exp bias table (layout prep only).
  2. Q projection (bf16, moving X^T) -> Q^T [64, S]; K/V projection
     (bf16, packed stationary, moving X_kv^T) -> kv [K^T rows 0:64 |
     V^T rows 64:128] over SKV compacted keys. DVE bias-add evacuations.
  3. Scores transposed S^T[k, q] = K^T.T Q^T, plain bf16 matmuls with
     64-deep contraction (cost is moving columns, so depth 64 is free).
  4. exp on ScalarE out of PSUM, scale=1/8 (absorbs 1/sqrt(H); weights
     stay unscaled), bias = -2 margin or -102 for pad keys; the margin
     cancels in the final divide.
  5. V' = [V | ones] rebuilt natural per key tile by TensorE transposes
     of kv rows 64:128 (identity corner at base partition 64). Attention
     runs in two query halves of 1024 so exp uses wide (1024-col)
     instructions while PSUM fits exactly: 2 score slots [128, 1024] +
     O'^T accumulator [65, 1024] + the projection ring = 8 banks. Row 64
     of O'^T is the softmax denominator; each half is evacuated as soon
     as it completes, overlapping the next half.
  6. Sample 1's entire prologue is deadline-scheduled into sample 0's
     attention iterations so TensorE/DVE/DMA work overlaps the exp
     stream without ever being emitted ahead of its producers.
  7. Host epilogue: O = O'[:64] / O'[64], transpose to [B, S, H].
"""

import functools

import ml_dtypes
import numpy as np

import concourse.bass as bass
import concourse.bacc as bacc
import concourse.mybir as mybir
import concourse.tile as tile
from concourse.bass_utils import run_bass_kernel_spmd
from concourse.masks import make_identity

F32 = mybir.dt.float32
BF16 = mybir.dt.bfloat16
AF = mybir.ActivationFunctionType
ALU = mybir.AluOpType

P = 128
B_PER_CORE = 2
S = 2048
W = 768
H = 64
NW = W // P      # 6 contraction chunks for the projections
NKT = S // P     # 16 key tiles uncompacted
NQC = S // 512   # 4 query chunks of 512
N_CORES = 8
PAD_BIAS = -100.0   # exp bias for pad keys (exp -> 0 exactly in bf16)
EXP_MARGIN = -2.0   # global exp bias margin (cancels in the divide)
QSCALE = 0.125      # 1/sqrt(H), applied as the exp scale

NP_BF16 = ml_dtypes.bfloat16


def _kv_chunks(skv):
    """PSUM-bank-sized (<=512 col) chunks covering the compacted keys."""
    edges = list(range(0, skv, 512)) + [skv]
    return list(zip(edges[:-1], edges[1:]))


def _emit_q_proj(nc, pools, b, qc):
    wq, bq, xt, qt, pps = (
        pools["wq"], pools["bq"], pools["xt"][b], pools["qt"][b], pools["pps"],
    )
    ps = pps.tile([P, 512], F32, tag="pps", name=f"pq_{b}_{qc}")
    for wc in range(NW):
        nc.tensor.matmul(
            ps[0:H, :],
            wq[:, wc, :],
            xt[:, wc, qc * 512 : (qc + 1) * 512],
            start=(wc == 0),
            stop=(wc == NW - 1),
        )
    nc.vector.tensor_scalar(
        qt[:, qc * 512 : (qc + 1) * 512], ps[0:H, :], bq, None, ALU.add
    )


def _emit_kv_proj(nc, pools, b, c0, c1):
    wkv, bkv, xkv, kv, pps = (
        pools["wkv"], pools["bkv"], pools["xkv"][b], pools["kv"][b], pools["pps"],
    )
    ps = pps.tile([P, 512], F32, tag="pps", name=f"pkv_{b}_{c0}")
    for wc in range(NW):
        nc.tensor.matmul(
            ps[:, 0 : c1 - c0],
            wkv[:, wc, :],
            xkv[:, wc, c0:c1],
            start=(wc == 0),
            stop=(wc == NW - 1),
        )
    nc.vector.tensor_scalar(kv[:, c0:c1], ps[:, 0 : c1 - c0], bkv, None, ALU.add)


def _emit_vtrans(nc, pools, b, j, nkt_kv):
    """Transpose kv rows 64:128 (V^T) for key-tile pair (2j, 2j+1) into
    natural bf16 V' tiles; the last pair may hold a single tile."""
    kv, vp, ident, pps = (
        pools["kv"][b], pools["vp"][b], pools["ident"], pools["pps"],
    )
    n = min(2, nkt_kv - 2 * j)
    pst = pps.tile([P, P], BF16, tag="pps", name=f"pvt_{b}_{j}")
    for i in range(n):
        kt = 2 * j + i
        nc.tensor.transpose(
            pst[:, i * H : (i + 1) * H],
            kv[H:P, kt * P : (kt + 1) * P],
            ident[H:P, H:P],
        )
    nc.vector.tensor_copy(
        vp[:, 2 * j : 2 * j + n, 0:H],
        pst[:, 0 : n * H].rearrange("p (i h) -> p i h", h=H),
    )


def _prologue_stages(nc, pools, b, skv, nkt_kv):
    stages = []
    for qc in range(NQC):
        stages.append(functools.partial(_emit_q_proj, nc, pools, b, qc))
    for c0, c1 in _kv_chunks(skv):
        stages.append(functools.partial(_emit_kv_proj, nc, pools, b, c0, c1))
    for j in range((nkt_kv + 1) // 2):
        stages.append(functools.partial(_emit_vtrans, nc, pools, b, j, nkt_kv))
    return stages


def _emit_attention(nc, pools, b, out_e, nkt_kv, interleave=()):
    """Score -> exp -> PV loops for sample b, split into two query halves
    of 1024 (PSUM: two 1024-wide score slots + one [65, 1024] output
    accumulator + the projection ring = exactly 8 banks). interleave is a
    flat list over the 2*nkt_kv iterations; interleave[it] thunks are
    emitted at the top of that iteration (the other sample's prologue,
    filling TensorE under the exp stream)."""
    qt, kv, vp, ebias = (
        pools["qt"][b], pools["kv"][b], pools["vp"][b], pools["ebias"][b],
    )
    sps_p, ptp, pso_p, oup = pools["sps"], pools["ptp"], pools["pso"], pools["oup"]

    # ones column of V' (row 64 of O'^T = softmax denominator)
    nc.gpsimd.memset(vp[:, :, H : H + 1], 1.0)

    HQ = S // 2
    for half in range(2):
        pso = pso_p.tile([H + 1, HQ], F32, tag="pso", name=f"pso{b}_{half}")
        for kt in range(nkt_kv):
            it = half * nkt_kv + kt
            for thunk in (interleave[it] if it < len(interleave) else ()):
                thunk()
            pt = ptp.tile([P, HQ], BF16, tag="pt", name=f"pt_{b}_{it}")
            sps = sps_p.tile([P, HQ], F32, tag="sps", name=f"ss_{b}_{it}")
            for qi in range(2):
                nc.tensor.matmul(
                    sps[:, qi * 512 : (qi + 1) * 512],
                    kv[0:H, kt * P : (kt + 1) * P],
                    qt[:, half * HQ + qi * 512 : half * HQ + (qi + 1) * 512],
                    start=True,
                    stop=True,
                )
            nc.scalar.activation(
                pt, sps, AF.Exp, bias=ebias[:, kt : kt + 1], scale=QSCALE
            )
            for qi in range(2):
                nc.tensor.matmul(
                    pso[:, qi * 512 : (qi + 1) * 512],
                    vp[:, kt, :],
                    pt[:, qi * 512 : (qi + 1) * 512],
                    start=(kt == 0),
                    stop=(kt == nkt_kv - 1),
                )
        # evacuate this half right away (overlaps the next half / sample)
        ou = oup.tile([H + 1, HQ], F32, tag="ou", name=f"ou{b}_{half}")
        for qi in range(2):
            sl = slice(qi * 512, (qi + 1) * 512)
            osl = slice(half * HQ + qi * 512, half * HQ + (qi + 1) * 512)
            nc.vector.tensor_copy(ou[:, sl], pso[:, sl])
            nc.sync.dma_start(out=out_e[b, :, osl], in_=ou[:, sl])


def _build(nc, tc, nkt_kv, xt_e, xkv_e, eb_e, wq_e, wkv_e, bq_e, bkv_e, out_e):
    skv = nkt_kv * P
    with (
        tc.tile_pool(name="const", bufs=1) as cpool,
        tc.tile_pool(name="xtp", bufs=2) as xtp,
        tc.tile_pool(name="xkvp", bufs=2) as xkvp,
        tc.tile_pool(name="qtp", bufs=2) as qtp,
        tc.tile_pool(name="kvp", bufs=2) as kvp,
        tc.tile_pool(name="vpp", bufs=2) as vpp,
        tc.tile_pool(name="ptp", bufs=2) as ptp,
        tc.tile_pool(name="oup", bufs=2) as oup,
        tc.tile_pool(name="ebp", bufs=2) as ebp,
        tc.tile_pool(name="sps", bufs=2, space="PSUM") as sps_p,  # 2x[128,1024]
        tc.tile_pool(name="pps", bufs=2, space="PSUM") as pps,
        tc.tile_pool(name="psop", bufs=1, space="PSUM") as pso_p,
    ):
        ident = cpool.tile([P, P], BF16, name="ident", tag="ident")
        make_identity(nc, ident)
        wq = cpool.tile([P, NW, H], BF16, name="wq", tag="wq")
        wkv = cpool.tile([P, NW, P], BF16, name="wkv", tag="wkv")
        bq = cpool.tile([H, 1], F32, name="bq", tag="bq")
        bkv = cpool.tile([P, 1], F32, name="bkv", tag="bkv")
        nc.gpsimd.dma_start(out=wq, in_=wq_e[:, :, :])
        nc.gpsimd.dma_start(out=wkv, in_=wkv_e[:, :, :])
        nc.gpsimd.dma_start(out=bq, in_=bq_e[:, :])
        nc.gpsimd.dma_start(out=bkv, in_=bkv_e[:, :])

        pools = {
            "ident": ident, "wq": wq, "wkv": wkv, "bq": bq, "bkv": bkv,
            "sps": sps_p, "pps": pps, "pso": pso_p, "ptp": ptp, "oup": oup,
            "xt": [], "xkv": [], "qt": [], "kv": [], "vp": [], "ebias": [],
        }
        for b in range(B_PER_CORE):
            eb = ebp.tile([P, nkt_kv], F32, tag="eb", name=f"eb{b}")
            nc.gpsimd.dma_start(out=eb, in_=eb_e[b])
            pools["ebias"].append(eb)
            pools["xt"].append(xtp.tile([P, NW, S], BF16, tag="xt", name=f"xt{b}"))
            pools["xkv"].append(
                xkvp.tile([P, NW, skv], BF16, tag="xkv", name=f"xkv{b}")
            )
            pools["qt"].append(qtp.tile([H, S], BF16, tag="qt", name=f"qt{b}"))
            pools["kv"].append(kvp.tile([P, skv], BF16, tag="kv", name=f"kv{b}"))
            pools["vp"].append(
                vpp.tile([P, nkt_kv, H + 1], BF16, tag="vp", name=f"vp{b}")
            )

        # input loads, sliced so the first projection groups start early;
        # sample 0 first.
        # Dispatch cost is ~0.7 us per DMA, serialized on the Sync queue,
        # so order follows consumption: both samples' ramp-critical chunks
        # (KV chunk 0 + Q chunks 0/1, i.e. query half 0) first, then the
        # rest (later KV chunks before the query-half-1 chunks).
        chunks = _kv_chunks(skv)
        crit = [("xkv", chunks[0]), ("xt", (0, 512)), ("xt", (512, 1024))]
        rest = [("xkv", c) for c in chunks[1:]] + [
            ("xt", (qc * 512, (qc + 1) * 512)) for qc in range(2, NQC)
        ]
        plan = [(b, k, c) for part in (crit, rest)
                for b in range(B_PER_CORE) for k, c in part]
        for b, kind, (c0, c1) in plan:
            dst = pools[kind][b]
            src_e = xt_e if kind == "xt" else xkv_e
            for wc in range(NW):
                nc.sync.dma_start(
                    out=dst[:, wc, c0:c1],
                    in_=src_e[b, :, wc, c0:c1],
                )

        # Deadline-scheduled interleave over the flat iteration space:
        # sample 0 runs iterations 0 .. 2*nkt-1, sample 1 runs 2*nkt ..
        # 4*nkt-1. A stage placed at iteration `it` is emitted before that
        # iteration's score matmuls (and before its PV block).
        s0 = _prologue_stages(nc, pools, 0, skv, nkt_kv)
        s1 = _prologue_stages(nc, pools, 1, skv, nkt_kv)
        nch = len(_kv_chunks(skv))
        npair = (nkt_kv + 1) // 2
        nit = 2 * nkt_kv

        # critical prologue of sample 0 up front: KV chunk 0 (first score
        # tiles), Q chunks 0/1 (query half 0), vtrans 0 (first PV pair)
        s0[NQC]()
        s0[0]()
        s0[1]()
        s0[NQC + nch]()

        # Greedy deadline scheduling into flat iteration slots, with
        # producer stages (projection chunks) placed before their
        # consumers (vtrans of the key tiles they produce).
        inter = [[] for _ in range(2 * nit)]
        used = [0] * (2 * nit)

        def place(st, lo, dl):
            lo = max(1, min(lo, dl, 2 * nit - 1))
            dl = min(dl, 2 * nit - 1)
            for cap in (1, 2, 99):
                for it in range(lo, dl + 1):
                    if used[it] < cap:
                        inter[it].append(st)
                        used[it] += 1
                        return it
            raise AssertionError("no slot")

        kvslot0 = {0: 0}  # b0 KV chunk -> emission slot (chunk 0 pre-loop)
        for c in range(1, nch):
            kvslot0[c] = place(s0[NQC + c], 1, min(4 * c, nit - 1))
        for qc in (2, 3):  # b0 Q chunks for its query half 1
            place(s0[qc], 1, nkt_kv)
        for j in range(1, npair):  # b0 vtrans j before b0 PV(kt=2j)
            src_c = (2 * j) * P // 512
            place(s0[NQC + nch + j], kvslot0.get(src_c, 0) + 1, 2 * j)

        kvslot1 = {}
        for c in range(nch):  # b1 KV chunks, before b1 ss(kt=4c)
            kvslot1[c] = place(
                s1[NQC + c], nkt_kv - 2, min(nit + 4 * c, 2 * nit - 1)
            )
        for qc in range(NQC):  # b1 Q chunks: halves 0/1 by deadline
            place(s1[qc], nkt_kv - 2, nit if qc < 2 else nit + nkt_kv)
        for j in range(npair):  # b1 vtrans j before b1 PV(kt=2j)
            src_c = (2 * j) * P // 512
            place(s1[NQC + nch + j], kvslot1[src_c] + 1, nit + 2 * j)
        _emit_attention(nc, pools, 0, out_e, nkt_kv, interleave=inter[:nit])
        _emit_attention(nc, pools, 1, out_e, nkt_kv, interleave=inter[nit:])


@functools.lru_cache(maxsize=2)
def build_nc(nkt_kv: int) -> bass.Bass:
    skv = nkt_kv * P
    nc = bacc.Bacc()
    xt_e = nc.declare_dram_parameter("xt", [B_PER_CORE, P, NW, S], BF16, isOutput=False)
    xkv_e = nc.declare_dram_parameter(
        "xkv", [B_PER_CORE, P, NW, skv], BF16, isOutput=False
    )
    eb_e = nc.declare_dram_parameter("eb", [B_PER_CORE, P, nkt_kv], F32, isOutput=False)
    wq_e = nc.declare_dram_parameter("wq", [P, NW, H], BF16, isOutput=False)
    wkv_e = nc.declare_dram_parameter("wkv", [P, NW, P], BF16, isOutput=False)
    bq_e = nc.declare_dram_parameter("bq", [H, 1], F32, isOutput=False)
    bkv_e = nc.declare_dram_parameter("bkv", [P, 1], F32, isOutput=False)
    out_e = nc.declare_dram_parameter("out", [B_PER_CORE, H + 1, S], F32, isOutput=True)

    with tile.TileContext(nc, pool_alloc_mode="queue") as tc:
        _build(nc, tc, nkt_kv, xt_e, xkv_e, eb_e, wq_e, wkv_e, bq_e, bkv_e, out_e)
    nc.finalize()
    return nc


def _host_prep(inputs):
    """Pack the full inputs into per-core DRAM layouts (layout/dtype/
    gather prep only; all arithmetic stays on device)."""
    inp = np.asarray(inputs["input"], dtype=np.float32)      # [16, S, W]
    msk = np.asarray(inputs["mask"], dtype=np.int32)         # [16, 1, S]
    B = inp.shape[0]

    # X^T[b, p, wc, s] = X[b, s, wc*128 + p]
    def pack_t(x):
        s = x.shape[1]
        return np.ascontiguousarray(
            x.transpose(0, 2, 1).reshape(B, NW, P, s).transpose(0, 2, 1, 3)
        ).astype(NP_BF16)

    xt = pack_t(inp)

    # compact the keys: per sample gather the valid positions, pad to an
    # even number of whole 128-key tiles (shared across cores: SPMD)
    valid = [np.nonzero(msk[b, 0])[0] for b in range(B)]
    nv_max = max(len(v) for v in valid)
    nkt_kv = min(-(-nv_max // P), NKT)
    skv = nkt_kv * P

    xkv_rows = np.zeros((B, skv, W), dtype=np.float32)
    eb = np.full((B, skv), PAD_BIAS, dtype=np.float32)
    for b in range(B):
        v = valid[b][:skv]
        xkv_rows[b, : len(v)] = inp[b, v]
        eb[b, : len(v)] = 0.0
    xkv = pack_t(xkv_rows)
    eb = (eb + EXP_MARGIN).reshape(B, nkt_kv, P).transpose(0, 2, 1)
    eb = np.ascontiguousarray(eb)

    wq_in = np.asarray(inputs["Wq"], dtype=np.float32)
    wk = np.asarray(inputs["Wk"], dtype=np.float32)
    wv = np.asarray(inputs["Wv"], dtype=np.float32)
    wq = np.ascontiguousarray(wq_in.reshape(NW, P, H).transpose(1, 0, 2)).astype(
        NP_BF16
    )
    wkv = np.concatenate([wk, wv], axis=1).reshape(NW, P, 2 * H).transpose(1, 0, 2)
    wkv = np.ascontiguousarray(wkv).astype(NP_BF16)

    bq = np.asarray(inputs["bq"], dtype=np.float32)[:, None]
    bkv = np.concatenate(
        [np.asarray(inputs["bk"]), np.asarray(inputs["bv"])]
    ).astype(np.float32)[:, None]
    return nkt_kv, xt, xkv, eb, wq, wkv, bq, bkv


def run(inputs, trace=False, **kwargs):
    nkt_kv, xt, xkv, eb, wq, wkv, bq, bkv = _host_prep(inputs)
    nc = build_nc(nkt_kv)
    in_maps = []
    for c in range(N_CORES):
        sl = slice(B_PER_CORE * c, B_PER_CORE * (c + 1))
        in_maps.append({
            "xt": xt[sl], "xkv": xkv[sl], "eb": eb[sl],
            "wq": wq, "wkv": wkv, "bq": bq, "bkv": bkv,
        })
    res = run_bass_kernel_spmd(nc, in_maps, list(range(N_CORES)), trace=trace, **kwargs)
    outs = np.concatenate(
        [res.results[i]["out"] for i in range(N_CORES)], axis=0
    )  # [16, 65, 2048]
    o = outs[:, :H, :] / outs[:, H : H + 1, :]
    return np.ascontiguousarray(o.transpose(0, 2, 1)).astype(np.float32), res


def kernel(**inputs):
    out, _ = run(inputs, trace=False)
    return out


# revision 21
# speedup vs baseline: 1.0387x; 1.0387x over previous
"""Trainium2 Bass kernel for a single attention head (nn_AttentionHead).

Problem: B=16, S=2048, W=768, H=64.
  Q = input @ Wq + bq ; K = input @ Wk + bk ; V = input @ Wv + bv
  scores = Q K^T / sqrt(H), key-padding mask, softmax, out = attn @ V.

Sharding: data-parallel over batch across 8 cores (2 samples per core).

Design (per core). Two cost facts drive it: TensorE matmul time
depends only on moving columns (contraction depth is free), and ScalarE
exp costs 0.83 ns per score-matrix column. Both scale with the number of
KEY tiles, and masked keys (about half: exp == 0 exactly) contribute
nothing — so the host compacts each sample's keys to the valid subset
(padded to whole 128-key tiles; pad keys get a -100 exp bias so they
are exactly zero, making compaction bit-equivalent).

  1. Host packs X^T bf16 [B, P, NW, S] for the Q pass, the compacted
     X_kv^T bf16 [B, P, NW, SKV] for the K/V pass, stationaries
     Wq / [Wk|Wv], biases, and the exp bias table (layout prep only).
  2. Q projection (bf16, moving X^T) -> Q^T [64, S]; K/V projection
     (bf16, packed stationary, moving X_kv^T) -> kv [K^T rows 0:64 |
     V^T rows 64:128] over SKV compacted keys. DVE bias-add evacuations.
  3. Scores transposed S^T[k, q] = K^T.T Q^T, plain bf16 matmuls with
     64-deep contraction (cost is moving columns, so depth 64 is free).
  4. exp on ScalarE out of PSUM, scale=1/8 (absorbs 1/sqrt(H); weights
     stay unscaled), bias = -2 margin or -102 for pad keys; the margin
     cancels in the final divide.
  5. V' = [V | ones] rebuilt natural per key tile by TensorE transposes
     of kv rows 64:128 (identity corner at base partition 64). Attention
     runs in two query halves of 1024 so exp uses wide (1024-col)
     instructions while PSUM fits exactly: 2 score slots [128, 1024] +
     O'^T accumulator [65, 1024] + the projection ring = 8 banks. Row 64
     of O'^T is the softmax denominator; each half is evacuated as soon
     as it completes, overlapping the next half.
  6. Sample 1's entire prologue is deadline-scheduled into sample 0's
     attention iterations so TensorE/DVE/DMA work overlaps the exp
     stream without ever being emitted ahead of its producers.
  7. Host epilogue: O = O'[:64] / O'[64], transpose to [B, S, H].
"""

import functools

import ml_dtypes
import numpy as np

import concourse.bass as bass
import concourse.bacc as bacc
import concourse.mybir as mybir
import concourse.tile as tile
from concourse.bass_utils import run_bass_kernel_spmd
from concourse.masks import make_identity

F32 = mybir.dt.float32
BF16 = mybir.dt.bfloat16
AF = mybir.ActivationFunctionType
ALU = mybir.AluOpType

P = 128
B_PER_CORE = 2
S = 2048
W = 768
H = 64
NW = W // P      # 6 contraction chunks for the projections
NKT = S // P     # 16 key tiles uncompacted
NQC = S // 512   # 4 query chunks of 512
N_CORES = 8
PAD_BIAS = -100.0   # exp bias for pad keys (exp -> 0 exactly in bf16)
EXP_MARGIN = -2.0   # global exp bias margin (cancels in the divide)
QSCALE = 0.125      # 1/sqrt(H), applied as the exp scale

NP_BF16 = ml_dtypes.bfloat16


def _kv_chunks(skv):
    """PSUM-bank-sized (<=512 col) chunks covering the compacted keys."""
    edges = list(range(0, skv, 512)) + [skv]
    return list(zip(edges[:-1], edges[1:]))


def _emit_q_proj(nc, pools, b, qc):
    wq, bq, xt, qt, pps = (
        pools["wq"], pools["bq"], pools["xt"][b], pools["qt"][b], pools["pps"],
    )
    ps = pps.tile([P, 512], F32, tag="pps", name=f"pq_{b}_{qc}")
    for wc in range(NW):
        nc.tensor.matmul(
            ps[0:H, :],
            wq[:, wc, :],
            xt[:, wc, qc * 512 : (qc + 1) * 512],
            start=(wc == 0),
            stop=(wc == NW - 1),
        )
    nc.vector.tensor_scalar(
        qt[:, qc * 512 : (qc + 1) * 512], ps[0:H, :], bq, None, ALU.add
    )


def _emit_kv_proj(nc, pools, b, c0, c1):
    wkv, bkv, xkv, kv, pps = (
        pools["wkv"], pools["bkv"], pools["xkv"][b], pools["kv"][b], pools["pps"],
    )
    ps = pps.tile([P, 512], F32, tag="pps", name=f"pkv_{b}_{c0}")
    for wc in range(NW):
        nc.tensor.matmul(
            ps[:, 0 : c1 - c0],
            wkv[:, wc, :],
            xkv[:, wc, c0:c1],
            start=(wc == 0),
            stop=(wc == NW - 1),
        )
    nc.vector.tensor_scalar(kv[:, c0:c1], ps[:, 0 : c1 - c0], bkv, None, ALU.add)


def _emit_vtrans(nc, pools, b, j, nkt_kv):
    """Transpose kv rows 64:128 (V^T) for key-tile pair (2j, 2j+1) into
    natural bf16 V' tiles; the last pair may hold a single tile."""
    kv, vp, ident, pps = (
        pools["kv"][b], pools["vp"][b], pools["ident"], pools["pps"],
    )
    n = min(2, nkt_kv - 2 * j)
    pst = pps.tile([P, P], BF16, tag="pps", name=f"pvt_{b}_{j}")
    for i in range(n):
        kt = 2 * j + i
        nc.tensor.transpose(
            pst[:, i * H : (i + 1) * H],
            kv[H:P, kt * P : (kt + 1) * P],
            ident[H:P, H:P],
        )
    nc.vector.tensor_copy(
        vp[:, 2 * j : 2 * j + n, 0:H],
        pst[:, 0 : n * H].rearrange("p (i h) -> p i h", h=H),
    )


def _prologue_stages(nc, pools, b, skv, nkt_kv):
    stages = []
    for qc in range(NQC):
        stages.append(functools.partial(_emit_q_proj, nc, pools, b, qc))
    for c0, c1 in _kv_chunks(skv):
        stages.append(functools.partial(_emit_kv_proj, nc, pools, b, c0, c1))
    for j in range((nkt_kv + 1) // 2):
        stages.append(functools.partial(_emit_vtrans, nc, pools, b, j, nkt_kv))
    return stages


def _emit_attention(nc, pools, b, out_e, nkt_kv, interleave=()):
    """Score -> exp -> PV loops for sample b, split into two query halves
    of 1024 (PSUM: two 1024-wide score slots + one [65, 1024] output
    accumulator + the projection ring = exactly 8 banks). interleave is a
    flat list over the 2*nkt_kv iterations; interleave[it] thunks are
    emitted at the top of that iteration (the other sample's prologue,
    filling TensorE under the exp stream)."""
    qt, kv, vp, ebias = (
        pools["qt"][b], pools["kv"][b], pools["vp"][b], pools["ebias"][b],
    )
    sps_p, ptp, pso_p, oup = pools["sps"], pools["ptp"], pools["pso"], pools["oup"]

    # ones column of V' (row 64 of O'^T = softmax denominator)
    nc.gpsimd.memset(vp[:, :, H : H + 1], 1.0)

    HQ = S // 2
    for half in range(2):
        pso = pso_p.tile([H + 1, HQ], F32, tag="pso", name=f"pso{b}_{half}")
        for kt in range(nkt_kv):
            it = half * nkt_kv + kt
            for thunk in (interleave[it] if it < len(interleave) else ()):
                thunk()
            pt = ptp.tile([P, HQ], BF16, tag="pt", name=f"pt_{b}_{it}")
            sps = sps_p.tile([P, HQ], F32, tag="sps", name=f"ss_{b}_{it}")
            for qi in range(2):
                nc.tensor.matmul(
                    sps[:, qi * 512 : (qi + 1) * 512],
                    kv[0:H, kt * P : (kt + 1) * P],
                    qt[:, half * HQ + qi * 512 : half * HQ + (qi + 1) * 512],
                    start=True,
                    stop=True,
                )
            nc.scalar.activation(
                pt, sps, AF.Exp, bias=ebias[:, kt : kt + 1], scale=QSCALE
            )
            for qi in range(2):
                nc.tensor.matmul(
                    pso[:, qi * 512 : (qi + 1) * 512],
                    vp[:, kt, :],
                    pt[:, qi * 512 : (qi + 1) * 512],
                    start=(kt == 0),
                    stop=(kt == nkt_kv - 1),
                )
        # evacuate this half right away (overlaps the next half / sample)
        ou = oup.tile([H + 1, HQ], F32, tag="ou", name=f"ou{b}_{half}")
        for qi in range(2):
            sl = slice(qi * 512, (qi + 1) * 512)
            osl = slice(half * HQ + qi * 512, half * HQ + (qi + 1) * 512)
            nc.vector.tensor_copy(ou[:, sl], pso[:, sl])
            nc.sync.dma_start(out=out_e[b, :, osl], in_=ou[:, sl])


def _build(nc, tc, nkt_kv, xt_e, xkv_e, eb_e, wq_e, wkv_e, bq_e, bkv_e, out_e):
    skv = nkt_kv * P
    with (
        tc.tile_pool(name="const", bufs=1) as cpool,
        tc.tile_pool(name="xtp", bufs=2) as xtp,
        tc.tile_pool(name="xkvp", bufs=2) as xkvp,
        tc.tile_pool(name="qtp", bufs=2) as qtp,
        tc.tile_pool(name="kvp", bufs=2) as kvp,
        tc.tile_pool(name="vpp", bufs=2) as vpp,
        tc.tile_pool(name="ptp", bufs=2) as ptp,
        tc.tile_pool(name="oup", bufs=2) as oup,
        tc.tile_pool(name="ebp", bufs=2) as ebp,
        tc.tile_pool(name="sps", bufs=2, space="PSUM") as sps_p,  # 2x[128,1024]
        tc.tile_pool(name="pps", bufs=2, space="PSUM") as pps,
        tc.tile_pool(name="psop", bufs=1, space="PSUM") as pso_p,
    ):
        ident = cpool.tile([P, P], BF16, name="ident", tag="ident")
        make_identity(nc, ident)
        wq = cpool.tile([P, NW, H], BF16, name="wq", tag="wq")
        wkv = cpool.tile([P, NW, P], BF16, name="wkv", tag="wkv")
        bq = cpool.tile([H, 1], F32, name="bq", tag="bq")
        bkv = cpool.tile([P, 1], F32, name="bkv", tag="bkv")
        nc.gpsimd.dma_start(out=wq, in_=wq_e[:, :, :])
        nc.gpsimd.dma_start(out=wkv, in_=wkv_e[:, :, :])
        nc.gpsimd.dma_start(out=bq, in_=bq_e[:, :])
        nc.gpsimd.dma_start(out=bkv, in_=bkv_e[:, :])

        pools = {
            "ident": ident, "wq": wq, "wkv": wkv, "bq": bq, "bkv": bkv,
            "sps": sps_p, "pps": pps, "pso": pso_p, "ptp": ptp, "oup": oup,
            "xt": [], "xkv": [], "qt": [], "kv": [], "vp": [], "ebias": [],
        }
        for b in range(B_PER_CORE):
            eb = ebp.tile([P, nkt_kv], F32, tag="eb", name=f"eb{b}")
            nc.gpsimd.dma_start(out=eb, in_=eb_e[b])
            pools["ebias"].append(eb)
            pools["xt"].append(xtp.tile([P, NW, S], BF16, tag="xt", name=f"xt{b}"))
            pools["xkv"].append(
                xkvp.tile([P, NW, skv], BF16, tag="xkv", name=f"xkv{b}")
            )
            pools["qt"].append(qtp.tile([H, S], BF16, tag="qt", name=f"qt{b}"))
            pools["kv"].append(kvp.tile([P, skv], BF16, tag="kv", name=f"kv{b}"))
            pools["vp"].append(
                vpp.tile([P, nkt_kv, H + 1], BF16, tag="vp", name=f"vp{b}")
            )

        # input loads, sliced so the first projection groups start early;
        # sample 0 first.
        # Load DMAs dispatch at ~0.7 us each, serialized on the Sync
        # queue; per-sample consumption order loses the least overall.
        chunks = _kv_chunks(skv)
        for b in range(B_PER_CORE):
            plan = [("xkv", chunks[0])] + [
                ("xt", (qc * 512, (qc + 1) * 512)) for qc in range(NQC)
            ] + [("xkv", c) for c in chunks[1:]]
            for kind, (c0, c1) in plan:
                dst = pools[kind][b]
                src_e = xt_e if kind == "xt" else xkv_e
                for wc in range(NW):
                    nc.sync.dma_start(
                        out=dst[:, wc, c0:c1],
                        in_=src_e[b, :, wc, c0:c1],
                    )

        # Deadline-scheduled interleave over the flat iteration space:
        # sample 0 runs iterations 0 .. 2*nkt-1, sample 1 runs 2*nkt ..
        # 4*nkt-1. A stage placed at iteration `it` is emitted before that
        # iteration's score matmuls (and before its PV block).
        s0 = _prologue_stages(nc, pools, 0, skv, nkt_kv)
        s1 = _prologue_stages(nc, pools, 1, skv, nkt_kv)
        nch = len(_kv_chunks(skv))
        npair = (nkt_kv + 1) // 2
        nit = 2 * nkt_kv

        # critical prologue of sample 0 up front: KV chunk 0 (first score
        # tiles), Q chunks 0/1 (query half 0), vtrans 0 (first PV pair)
        s0[NQC]()
        s0[0]()
        s0[1]()
        s0[NQC + nch]()

        # Greedy deadline scheduling into flat iteration slots, with
        # producer stages (projection chunks) placed before their
        # consumers (vtrans of the key tiles they produce).
        inter = [[] for _ in range(2 * nit)]
        used = [0] * (2 * nit)

        def place(st, lo, dl):
            lo = max(1, min(lo, dl, 2 * nit - 1))
            dl = min(dl, 2 * nit - 1)
            for cap in (1, 2, 99):
                for it in range(lo, dl + 1):
                    if used[it] < cap:
                        inter[it].append(st)
                        used[it] += 1
                        return it
            raise AssertionError("no slot")

        kvslot0 = {0: 0}  # b0 KV chunk -> emission slot (chunk 0 pre-loop)
        for c in range(1, nch):
            kvslot0[c] = place(s0[NQC + c], 1, min(4 * c, nit - 1))
        for qc in (2, 3):  # b0 Q chunks for its query half 1
            place(s0[qc], 1, nkt_kv)
        for j in range(1, npair):  # b0 vtrans j before b0 PV(kt=2j)
            src_c = (2 * j) * P // 512
            place(s0[NQC + nch + j], kvslot0.get(src_c, 0) + 1, 2 * j)

        kvslot1 = {}
        for c in range(nch):  # b1 KV chunks, before b1 ss(kt=4c)
            kvslot1[c] = place(
                s1[NQC + c], nkt_kv - 2, min(nit + 4 * c, 2 * nit - 1)
            )
        for qc in range(NQC):  # b1 Q chunks: halves 0/1 by deadline
            place(s1[qc], nkt_kv - 2, nit if qc < 2 else nit + nkt_kv)
        for j in range(npair):  # b1 vtrans j before b1 PV(kt=2j)
            src_c = (2 * j) * P // 512
            place(s1[NQC + nch + j], kvslot1[src_c] + 1, nit + 2 * j)
        _emit_attention(nc, pools, 0, out_e, nkt_kv, interleave=inter[:nit])
        _emit_attention(nc, pools, 1, out_e, nkt_kv, interleave=inter[nit:])


@functools.lru_cache(maxsize=2)
def build_nc(nkt_kv: int) -> bass.Bass:
    skv = nkt_kv * P
    nc = bacc.Bacc()
    xt_e = nc.declare_dram_parameter("xt", [B_PER_CORE, P, NW, S], BF16, isOutput=False)
    xkv_e = nc.declare_dram_parameter(
        "xkv", [B_PER_CORE, P, NW, skv], BF16, isOutput=False
    )
    eb_e = nc.declare_dram_parameter("eb", [B_PER_CORE, P, nkt_kv], F32, isOutput=False)
    wq_e = nc.declare_dram_parameter("wq", [P, NW, H], BF16, isOutput=False)
    wkv_e = nc.declare_dram_parameter("wkv", [P, NW, P], BF16, isOutput=False)
    bq_e = nc.declare_dram_parameter("bq", [H, 1], F32, isOutput=False)
    bkv_e = nc.declare_dram_parameter("bkv", [P, 1], F32, isOutput=False)
    out_e = nc.declare_dram_parameter("out", [B_PER_CORE, H + 1, S], F32, isOutput=True)

    with tile.TileContext(nc, pool_alloc_mode="queue") as tc:
        _build(nc, tc, nkt_kv, xt_e, xkv_e, eb_e, wq_e, wkv_e, bq_e, bkv_e, out_e)
    nc.finalize()
    return nc


def _host_prep(inputs):
    """Pack the full inputs into per-core DRAM layouts (layout/dtype/
    gather prep only; all arithmetic stays on device)."""
    inp = np.asarray(inputs["input"], dtype=np.float32)      # [16, S, W]
    msk = np.asarray(inputs["mask"], dtype=np.int32)         # [16, 1, S]
    B = inp.shape[0]

    # X^T[b, p, wc, s] = X[b, s, wc*128 + p]
    def pack_t(x):
        s = x.shape[1]
        return np.ascontiguousarray(
            x.transpose(0, 2, 1).reshape(B, NW, P, s).transpose(0, 2, 1, 3)
        ).astype(NP_BF16)

    xt = pack_t(inp)

    # compact the keys: per sample gather the valid positions, pad to an
    # even number of whole 128-key tiles (shared across cores: SPMD)
    valid = [np.nonzero(msk[b, 0])[0] for b in range(B)]
    nv_max = max(len(v) for v in valid)
    nkt_kv = min(-(-nv_max // P), NKT)
    skv = nkt_kv * P

    xkv_rows = np.zeros((B, skv, W), dtype=np.float32)
    eb = np.full((B, skv), PAD_BIAS, dtype=np.float32)
    for b in range(B):
        v = valid[b][:skv]
        xkv_rows[b, : len(v)] = inp[b, v]
        eb[b, : len(v)] = 0.0
    xkv = pack_t(xkv_rows)
    eb = (eb + EXP_MARGIN).reshape(B, nkt_kv, P).transpose(0, 2, 1)
    eb = np.ascontiguousarray(eb)

    wq_in = np.asarray(inputs["Wq"], dtype=np.float32)
    wk = np.asarray(inputs["Wk"], dtype=np.float32)
    wv = np.asarray(inputs["Wv"], dtype=np.float32)
    wq = np.ascontiguousarray(wq_in.reshape(NW, P, H).transpose(1, 0, 2)).astype(
        NP_BF16
    )
    wkv = np.concatenate([wk, wv], axis=1).reshape(NW, P, 2 * H).transpose(1, 0, 2)
    wkv = np.ascontiguousarray(wkv).astype(NP_BF16)

    bq = np.asarray(inputs["bq"], dtype=np.float32)[:, None]
    bkv = np.concatenate(
        [np.asarray(inputs["bk"]), np.asarray(inputs["bv"])]
    ).astype(np.float32)[:, None]
    return nkt_kv, xt, xkv, eb, wq, wkv, bq, bkv


def run(inputs, trace=False, **kwargs):
    nkt_kv, xt, xkv, eb, wq, wkv, bq, bkv = _host_prep(inputs)
    nc = build_nc(nkt_kv)
    in_maps = []
    for c in range(N_CORES):
        sl = slice(B_PER_CORE * c, B_PER_CORE * (c + 1))
        in_maps.append({
            "xt": xt[sl], "xkv": xkv[sl], "eb": eb[sl],
            "wq": wq, "wkv": wkv, "bq": bq, "bkv": bkv,
        })
    res = run_bass_kernel_spmd(nc, in_maps, list(range(N_CORES)), trace=trace, **kwargs)
    outs = np.concatenate(
        [res.results[i]["out"] for i in range(N_CORES)], axis=0
    )  # [16, 65, 2048]
    o = outs[:, :H, :] / outs[:, H : H + 1, :]
    return np.ascontiguousarray(o.transpose(0, 2, 1)).astype(np.float32), res


def kernel(**inputs):
    out, _ = run(inputs, trace=False)
    return out


# revision 22
# speedup vs baseline: 1.0956x; 1.0547x over previous
"""Trainium2 Bass kernel for a single attention head (nn_AttentionHead).

Problem: B=16, S=2048, W=768, H=64.
  Q = input @ Wq + bq ; K = input @ Wk + bk ; V = input @ Wv + bv
  scores = Q K^T / sqrt(H), key-padding mask, softmax, out = attn @ V.

Sharding: data-parallel over batch across 8 cores (2 samples per core).

Design (per core). Two cost facts drive it: TensorE matmul time
depends only on moving columns (contraction depth is free), and ScalarE
exp costs 0.83 ns per score-matrix column. Both scale with the number of
KEY tiles, and masked keys (about half: exp == 0 exactly) contribute
nothing — so the host compacts each sample's keys to the valid subset
(padded to whole 128-key tiles; pad keys get a -100 exp bias so they
are exactly zero, making compaction bit-equivalent).

  1. Host packs X^T bf16 [B, P, NW, S] for the Q pass, the compacted
     X_kv^T bf16 [B, P, NW, SKV] for the K/V pass, stationaries
     Wq / [Wk|Wv], biases, and the exp bias table (layout prep only).
  2. Q projection (bf16, moving X^T) -> Q^T [64, S]; K/V projection
     (bf16, packed stationary, moving X_kv^T) -> kv [K^T rows 0:64 |
     V^T rows 64:128] over SKV compacted keys. DVE bias-add evacuations.
  3. Scores transposed S^T[k, q] = K^T.T Q^T, plain bf16 matmuls with
     64-deep contraction (cost is moving columns, so depth 64 is free).
  4. exp on ScalarE out of PSUM, scale=1/8 (absorbs 1/sqrt(H); weights
     stay unscaled), bias = -2 margin or -102 for pad keys; the margin
     cancels in the final divide.
  5. V' = [V | ones] rebuilt natural per key tile by TensorE transposes
     of kv rows 64:128 (identity corner at base partition 64). Attention
     runs in two query halves of 1024 so exp uses wide (1024-col)
     instructions while PSUM fits exactly: 2 score slots [128, 1024] +
     O'^T accumulator [65, 1024] + the projection ring = 8 banks. Row 64
     of O'^T is the softmax denominator; each half is evacuated as soon
     as it completes, overlapping the next half.
  6. Sample 1's entire prologue is deadline-scheduled into sample 0's
     attention iterations so TensorE/DVE/DMA work overlaps the exp
     stream without ever being emitted ahead of its producers.
  7. Host epilogue: O = O'[:64] / O'[64], transpose to [B, S, H].
"""

import functools

import ml_dtypes
import numpy as np

import concourse.bass as bass
import concourse.bacc as bacc
import concourse.mybir as mybir
import concourse.tile as tile
from concourse.bass_utils import run_bass_kernel_spmd
from concourse.masks import make_identity

F32 = mybir.dt.float32
BF16 = mybir.dt.bfloat16
AF = mybir.ActivationFunctionType
ALU = mybir.AluOpType

P = 128
B_PER_CORE = 2
S = 2048
W = 768
H = 64
NW = W // P      # 6 contraction chunks for the projections
NKT = S // P     # 16 key tiles uncompacted
NQC = S // 512   # 4 query chunks of 512
N_CORES = 8
PAD_BIAS = -100.0   # exp bias for pad keys (exp -> 0 exactly in bf16)
EXP_MARGIN = -2.0   # global exp bias margin (cancels in the divide)
QSCALE = 0.125      # 1/sqrt(H), applied as the exp scale

NP_BF16 = ml_dtypes.bfloat16


def _kv_chunks(skv):
    """PSUM-bank-sized (<=512 col) chunks covering the compacted keys."""
    edges = list(range(0, skv, 512)) + [skv]
    return list(zip(edges[:-1], edges[1:]))


def _emit_q_proj(nc, pools, b, qc):
    wq, bq, xt, qt, pps = (
        pools["wq"], pools["bq"], pools["xt"][b], pools["qt"][b], pools["pps"],
    )
    ps = pps.tile([P, 512], F32, tag="pps", name=f"pq_{b}_{qc}")
    for wc in range(NW):
        nc.tensor.matmul(
            ps[0:H, :],
            wq[:, wc, :],
            xt[:, wc, qc * 512 : (qc + 1) * 512],
            start=(wc == 0),
            stop=(wc == NW - 1),
        )
    nc.vector.tensor_scalar(
        qt[:, qc * 512 : (qc + 1) * 512], ps[0:H, :], bq, None, ALU.add
    )


def _emit_kv_proj(nc, pools, b, c0, c1):
    wkv, bkv, xkv, kv, pps = (
        pools["wkv"], pools["bkv"], pools["xkv"][b], pools["kv"][b], pools["pps"],
    )
    ps = pps.tile([P, 512], F32, tag="pps", name=f"pkv_{b}_{c0}")
    for wc in range(NW):
        nc.tensor.matmul(
            ps[:, 0 : c1 - c0],
            wkv[:, wc, :],
            xkv[:, wc, c0:c1],
            start=(wc == 0),
            stop=(wc == NW - 1),
        )
    nc.vector.tensor_scalar(kv[:, c0:c1], ps[:, 0 : c1 - c0], bkv, None, ALU.add)


def _emit_vtrans(nc, pools, b, j, nkt_kv):
    """Transpose kv rows 64:128 (V^T) for key-tile pair (2j, 2j+1) into
    natural bf16 V' tiles; the last pair may hold a single tile."""
    kv, vp, ident, pps = (
        pools["kv"][b], pools["vp"][b], pools["ident"], pools["pps"],
    )
    n = min(2, nkt_kv - 2 * j)
    pst = pps.tile([P, P], BF16, tag="pps", name=f"pvt_{b}_{j}")
    for i in range(n):
        kt = 2 * j + i
        nc.tensor.transpose(
            pst[:, i * H : (i + 1) * H],
            kv[H:P, kt * P : (kt + 1) * P],
            ident[H:P, H:P],
        )
    nc.vector.tensor_copy(
        vp[:, 2 * j : 2 * j + n, 0:H],
        pst[:, 0 : n * H].rearrange("p (i h) -> p i h", h=H),
    )


def _prologue_stages(nc, pools, b, skv, nkt_kv):
    stages = []
    for qc in range(NQC):
        stages.append(functools.partial(_emit_q_proj, nc, pools, b, qc))
    for c0, c1 in _kv_chunks(skv):
        stages.append(functools.partial(_emit_kv_proj, nc, pools, b, c0, c1))
    for j in range((nkt_kv + 1) // 2):
        stages.append(functools.partial(_emit_vtrans, nc, pools, b, j, nkt_kv))
    return stages


def _emit_attention(nc, pools, b, out_e, nkt_kv, interleave=()):
    """Score -> exp -> PV loops for sample b, split into two query halves
    of 1024 (PSUM: two 1024-wide score slots + one [65, 1024] output
    accumulator + the projection ring = exactly 8 banks). interleave is a
    flat list over the 2*nkt_kv iterations; interleave[it] thunks are
    emitted at the top of that iteration (the other sample's prologue,
    filling TensorE under the exp stream)."""
    qt, kv, vp, ebias = (
        pools["qt"][b], pools["kv"][b], pools["vp"][b], pools["ebias"][b],
    )
    sps_p, ptp, pso_p, oup = pools["sps"], pools["ptp"], pools["pso"], pools["oup"]

    # ones column of V' (row 64 of O'^T = softmax denominator)
    nc.gpsimd.memset(vp[:, :, H : H + 1], 1.0)

    HQ = S // 2
    for half in range(2):
        pso = pso_p.tile([H + 1, HQ], F32, tag="pso", name=f"pso{b}_{half}")
        for kt in range(nkt_kv):
            it = half * nkt_kv + kt
            for thunk in (interleave[it] if it < len(interleave) else ()):
                thunk()
            pt = ptp.tile([P, HQ], BF16, tag="pt", name=f"pt_{b}_{it}")
            sps = sps_p.tile([P, HQ], F32, tag="sps", name=f"ss_{b}_{it}")
            for qi in range(2):
                nc.tensor.matmul(
                    sps[:, qi * 512 : (qi + 1) * 512],
                    kv[0:H, kt * P : (kt + 1) * P],
                    qt[:, half * HQ + qi * 512 : half * HQ + (qi + 1) * 512],
                    start=True,
                    stop=True,
                )
            nc.scalar.activation(
                pt, sps, AF.Exp, bias=ebias[:, kt : kt + 1], scale=QSCALE
            )
            for qi in range(2):
                nc.tensor.matmul(
                    pso[:, qi * 512 : (qi + 1) * 512],
                    vp[:, kt, :],
                    pt[:, qi * 512 : (qi + 1) * 512],
                    start=(kt == 0),
                    stop=(kt == nkt_kv - 1),
                )
        # evacuate this half right away (overlaps the next half / sample)
        ou = oup.tile([H + 1, HQ], F32, tag="ou", name=f"ou{b}_{half}")
        for qi in range(2):
            sl = slice(qi * 512, (qi + 1) * 512)
            osl = slice(half * HQ + qi * 512, half * HQ + (qi + 1) * 512)
            nc.vector.tensor_copy(ou[:, sl], pso[:, sl])
            nc.gpsimd.dma_start(out=out_e[b, :, osl], in_=ou[:, sl])


def _build(nc, tc, nkt_kv, xt_e, xkv_e, eb_e, wq_e, wkv_e, bq_e, bkv_e, out_e):
    skv = nkt_kv * P
    with (
        tc.tile_pool(name="const", bufs=1) as cpool,
        tc.tile_pool(name="xtp", bufs=2) as xtp,
        tc.tile_pool(name="xkvp", bufs=2) as xkvp,
        tc.tile_pool(name="qtp", bufs=2) as qtp,
        tc.tile_pool(name="kvp", bufs=2) as kvp,
        tc.tile_pool(name="vpp", bufs=2) as vpp,
        tc.tile_pool(name="ptp", bufs=2) as ptp,
        tc.tile_pool(name="oup", bufs=2) as oup,
        tc.tile_pool(name="ebp", bufs=2) as ebp,
        tc.tile_pool(name="sps", bufs=2, space="PSUM") as sps_p,  # 2x[128,1024]
        tc.tile_pool(name="pps", bufs=2, space="PSUM") as pps,
        tc.tile_pool(name="psop", bufs=1, space="PSUM") as pso_p,
    ):
        ident = cpool.tile([P, P], BF16, name="ident", tag="ident")
        make_identity(nc, ident)
        wq = cpool.tile([P, NW, H], BF16, name="wq", tag="wq")
        wkv = cpool.tile([P, NW, P], BF16, name="wkv", tag="wkv")
        bq = cpool.tile([H, 1], F32, name="bq", tag="bq")
        bkv = cpool.tile([P, 1], F32, name="bkv", tag="bkv")
        nc.gpsimd.dma_start(out=wq, in_=wq_e[:, :, :])
        nc.gpsimd.dma_start(out=wkv, in_=wkv_e[:, :, :])
        nc.gpsimd.dma_start(out=bq, in_=bq_e[:, :])
        nc.gpsimd.dma_start(out=bkv, in_=bkv_e[:, :])

        pools = {
            "ident": ident, "wq": wq, "wkv": wkv, "bq": bq, "bkv": bkv,
            "sps": sps_p, "pps": pps, "pso": pso_p, "ptp": ptp, "oup": oup,
            "xt": [], "xkv": [], "qt": [], "kv": [], "vp": [], "ebias": [],
        }
        for b in range(B_PER_CORE):
            eb = ebp.tile([P, nkt_kv], F32, tag="eb", name=f"eb{b}")
            nc.gpsimd.dma_start(out=eb, in_=eb_e[b])
            pools["ebias"].append(eb)
            pools["xt"].append(xtp.tile([P, NW, S], BF16, tag="xt", name=f"xt{b}"))
            pools["xkv"].append(
                xkvp.tile([P, NW, skv], BF16, tag="xkv", name=f"xkv{b}")
            )
            pools["qt"].append(qtp.tile([H, S], BF16, tag="qt", name=f"qt{b}"))
            pools["kv"].append(kvp.tile([P, skv], BF16, tag="kv", name=f"kv{b}"))
            pools["vp"].append(
                vpp.tile([P, nkt_kv, H + 1], BF16, tag="vp", name=f"vp{b}")
            )

        # input loads, sliced so the first projection groups start early;
        # sample 0 first.
        # Load DMAs dispatch at ~0.7 us each, serialized on the Sync
        # queue; per-sample consumption order loses the least overall.
        plans = [
            # sample 0: KV chunk 0 first (first score tiles), fine-grained
            # Q chunks (ramp), then the remaining keys as one DMA per wc
            [("xkv", (0, min(512, skv)))]
            + [("xt", (qc * 512, (qc + 1) * 512)) for qc in range(NQC)]
            + ([("xkv", (512, skv))] if skv > 512 else []),
            # sample 1: halves (needed ~30+ us in; fewer dispatches win)
            [("xkv", (0, min(512, skv)))]
            + [("xt", (0, S // 2)), ("xt", (S // 2, S))]
            + ([("xkv", (512, skv))] if skv > 512 else []),
        ]
        for b in range(B_PER_CORE):
            for kind, (c0, c1) in plans[b]:
                dst = pools[kind][b]
                src_e = xt_e if kind == "xt" else xkv_e
                for wc in range(NW):
                    nc.sync.dma_start(
                        out=dst[:, wc, c0:c1],
                        in_=src_e[b, :, wc, c0:c1],
                    )

        # Deadline-scheduled interleave over the flat iteration space:
        # sample 0 runs iterations 0 .. 2*nkt-1, sample 1 runs 2*nkt ..
        # 4*nkt-1. A stage placed at iteration `it` is emitted before that
        # iteration's score matmuls (and before its PV block).
        s0 = _prologue_stages(nc, pools, 0, skv, nkt_kv)
        s1 = _prologue_stages(nc, pools, 1, skv, nkt_kv)
        nch = len(_kv_chunks(skv))
        npair = (nkt_kv + 1) // 2
        nit = 2 * nkt_kv

        # critical prologue of sample 0 up front: KV chunk 0 (first score
        # tiles), Q chunks 0/1 (query half 0), vtrans 0 (first PV pair)
        s0[NQC]()
        s0[0]()
        s0[1]()
        s0[NQC + nch]()

        # Greedy deadline scheduling into flat iteration slots, with
        # producer stages (projection chunks) placed before their
        # consumers (vtrans of the key tiles they produce).
        inter = [[] for _ in range(2 * nit)]
        used = [0] * (2 * nit)

        def place(st, lo, dl):
            lo = max(1, min(lo, dl, 2 * nit - 1))
            dl = min(dl, 2 * nit - 1)
            for cap in (1, 2, 99):
                for it in range(lo, dl + 1):
                    if used[it] < cap:
                        inter[it].append(st)
                        used[it] += 1
                        return it
            raise AssertionError("no slot")

        kvslot0 = {0: 0}  # b0 KV chunk -> emission slot (chunk 0 pre-loop)
        for c in range(1, nch):
            kvslot0[c] = place(s0[NQC + c], 1, min(4 * c, nit - 1))
        for qc in (2, 3):  # b0 Q chunks for its query half 1
            place(s0[qc], 1, nkt_kv)
        for j in range(1, npair):  # b0 vtrans j before b0 PV(kt=2j)
            src_c = (2 * j) * P // 512
            place(s0[NQC + nch + j], kvslot0.get(src_c, 0) + 1, 2 * j)

        kvslot1 = {}
        for c in range(nch):  # b1 KV chunks, before b1 ss(kt=4c)
            kvslot1[c] = place(
                s1[NQC + c], nkt_kv - 2, min(nit + 4 * c, 2 * nit - 1)
            )
        for qc in range(NQC):  # b1 Q chunks: halves 0/1 by deadline
            place(s1[qc], nkt_kv - 2, nit if qc < 2 else nit + nkt_kv)
        for j in range(npair):  # b1 vtrans j before b1 PV(kt=2j)
            src_c = (2 * j) * P // 512
            place(s1[NQC + nch + j], kvslot1[src_c] + 1, nit + 2 * j)
        _emit_attention(nc, pools, 0, out_e, nkt_kv, interleave=inter[:nit])
        _emit_attention(nc, pools, 1, out_e, nkt_kv, interleave=inter[nit:])


@functools.lru_cache(maxsize=2)
def build_nc(nkt_kv: int) -> bass.Bass:
    skv = nkt_kv * P
    nc = bacc.Bacc()
    xt_e = nc.declare_dram_parameter("xt", [B_PER_CORE, P, NW, S], BF16, isOutput=False)
    xkv_e = nc.declare_dram_parameter(
        "xkv", [B_PER_CORE, P, NW, skv], BF16, isOutput=False
    )
    eb_e = nc.declare_dram_parameter("eb", [B_PER_CORE, P, nkt_kv], F32, isOutput=False)
    wq_e = nc.declare_dram_parameter("wq", [P, NW, H], BF16, isOutput=False)
    wkv_e = nc.declare_dram_parameter("wkv", [P, NW, P], BF16, isOutput=False)
    bq_e = nc.declare_dram_parameter("bq", [H, 1], F32, isOutput=False)
    bkv_e = nc.declare_dram_parameter("bkv", [P, 1], F32, isOutput=False)
    out_e = nc.declare_dram_parameter("out", [B_PER_CORE, H + 1, S], F32, isOutput=True)

    with tile.TileContext(nc, pool_alloc_mode="queue") as tc:
        _build(nc, tc, nkt_kv, xt_e, xkv_e, eb_e, wq_e, wkv_e, bq_e, bkv_e, out_e)
    nc.finalize()
    return nc


def _host_prep(inputs):
    """Pack the full inputs into per-core DRAM layouts (layout/dtype/
    gather prep only; all arithmetic stays on device)."""
    inp = np.asarray(inputs["input"], dtype=np.float32)      # [16, S, W]
    msk = np.asarray(inputs["mask"], dtype=np.int32)         # [16, 1, S]
    B = inp.shape[0]

    # X^T[b, p, wc, s] = X[b, s, wc*128 + p]
    def pack_t(x):
        s = x.shape[1]
        return np.ascontiguousarray(
            x.transpose(0, 2, 1).reshape(B, NW, P, s).transpose(0, 2, 1, 3)
        ).astype(NP_BF16)

    xt = pack_t(inp)

    # compact the keys: per sample gather the valid positions, pad to an
    # even number of whole 128-key tiles (shared across cores: SPMD)
    valid = [np.nonzero(msk[b, 0])[0] for b in range(B)]
    nv_max = max(len(v) for v in valid)
    nkt_kv = min(-(-nv_max // P), NKT)
    skv = nkt_kv * P

    xkv_rows = np.zeros((B, skv, W), dtype=np.float32)
    eb = np.full((B, skv), PAD_BIAS, dtype=np.float32)
    for b in range(B):
        v = valid[b][:skv]
        xkv_rows[b, : len(v)] = inp[b, v]
        eb[b, : len(v)] = 0.0
    xkv = pack_t(xkv_rows)
    eb = (eb + EXP_MARGIN).reshape(B, nkt_kv, P).transpose(0, 2, 1)
    eb = np.ascontiguousarray(eb)

    wq_in = np.asarray(inputs["Wq"], dtype=np.float32)
    wk = np.asarray(inputs["Wk"], dtype=np.float32)
    wv = np.asarray(inputs["Wv"], dtype=np.float32)
    wq = np.ascontiguousarray(wq_in.reshape(NW, P, H).transpose(1, 0, 2)).astype(
        NP_BF16
    )
    wkv = np.concatenate([wk, wv], axis=1).reshape(NW, P, 2 * H).transpose(1, 0, 2)
    wkv = np.ascontiguousarray(wkv).astype(NP_BF16)

    bq = np.asarray(inputs["bq"], dtype=np.float32)[:, None]
    bkv = np.concatenate(
        [np.asarray(inputs["bk"]), np.asarray(inputs["bv"])]
    ).astype(np.float32)[:, None]
    return nkt_kv, xt, xkv, eb, wq, wkv, bq, bkv


def run(inputs, trace=False, **kwargs):
    nkt_kv, xt, xkv, eb, wq, wkv, bq, bkv = _host_prep(inputs)
    nc = build_nc(nkt_kv)
    in_maps = []
    for c in range(N_CORES):
        sl = slice(B_PER_CORE * c, B_PER_CORE * (c + 1))
        in_maps.append({
            "xt": xt[sl], "xkv": xkv[sl], "eb": eb[sl],
            "wq": wq, "wkv": wkv, "bq": bq, "bkv": bkv,
        })
    res = run_bass_kernel_spmd(nc, in_maps, list(range(N_CORES)), trace=trace, **kwargs)
    outs = np.concatenate(
        [res.results[i]["out"] for i in range(N_CORES)], axis=0
    )  # [16, 65, 2048]
    o = outs[:, :H, :] / outs[:, H : H + 1, :]
    return np.ascontiguousarray(o.transpose(0, 2, 1)).astype(np.float32), res


def kernel(**inputs):
    out, _ = run(inputs, trace=False)
    return out
